# revision 57
# baseline (speedup 1.0000x reference)
"""MoE (63 routed experts top-7 + 1 shared expert) Trainium2 kernel.

Strategy: expert-parallel sparse dispatch. The router (softmax + top-k,
~0.3% of FLOPs) runs on host; tokens are gathered expert-major into
fixed-capacity weight slots, which are distributed across 8 NeuronCores.
Each core runs an identical (SPMD) Bass program of 9 slots x 1024
tokens: 8 routed-expert slots + 1 shared-expert slot.

Precision plan (validated against a bit-accurate numpy simulation and
measured on HW at rel-err 1.68e-2 vs the 2e-2 gate):
  * ALL matmuls run fp8e4m3 with DoubleRow (2 k-tiles per instruction,
    2x Tensor throughput).  Weights are pre-scaled x64 and activations
    x16 on host to stay out of the e4m3 subnormal range; the product
    scale is divided back out via the activation/vector units' scale
    operand.  Gate-damping (sum g_i << 1) keeps the routed error small;
    the ungated shared expert dominates the (passing) error budget.

Per slot: Linear -> exact GELU (Scalar engine, bias+scale ports) ->
Linear -> bias+scale on the Vector engine -> bf16 store. Everything is
feature-major (features on partitions, tokens on the free dim) so
weights need no transpose. Engine/queue assignment keeps the Tensor
engine saturated: x loads on SWDGE, weight loads on the Sync HWDGE
queue (6-deep pools so they prefetch a full slot ahead), y stores on
the Scalar HWDGE queue, layer-2 drain on the otherwise-idle Vector
engine. Outputs are gathered and gate-weighted back on host in the
reference's exact accumulation order.
"""

import math
import sys

sys.path.insert(0, "/opt/trn_rl_repo")

import numpy as np

D = 1280          # model dim
I = 1280          # expert inter dim
EXPERTS = 63      # routed experts
TOPK = 7          # routed top-k
CAP = 1024        # tokens per weight slot
CHUNK = 512       # tokens per matmul (PSUM bank limit)
KT = D // 128     # 10 contraction tiles
NCORES = 8

# fp8 scaling: weights x64 and activations x16 keep values out of the e4m3
# subnormal range (|v| < 2^-6); the product scale 1/1024 (layer 1) and 1/64
# (layer 2) is folded into the activation's scale port.
WSCALE = 64.0
XSCALE = 16.0

_PROGRAM_CACHE = {}


# ----------------------------------------------------------------- router

def _route(x2d, wr, br):
    """f32 softmax + top-k, matching jax.nn.softmax / jax.lax.top_k."""
    logits = (x2d @ wr + br).astype(np.float32)
    logits -= logits.max(-1, keepdims=True)
    np.exp(logits, out=logits)
    aff = logits / logits.sum(-1, keepdims=True)
    idx = np.argsort(-aff, axis=-1, kind="stable")[:, :TOPK]
    vals = np.take_along_axis(aff, idx, axis=-1)
    return idx.astype(np.int32), vals.astype(np.float32)


def _build_plan(T, idx):
    """Routed (token, expert) pairs packed expert-major into <=CAP-token
    pieces, sorted by size and grouped 8-at-a-time into slots: cell
    (core, j) holds pieces[8j + core], and slot j's capacity is only its
    group max (matmul free-dims are arbitrary, so slots need not be
    CAP-wide).  Sorting descending minimizes sum(caps) = compute.
    Shared tokens -> 8 cores x S16 fp16 slots of CAP."""
    flat = idx.ravel()
    order = np.argsort(flat, kind="stable")          # expert-major slot order
    tok_of = (order // TOPK).astype(np.int64)
    counts = np.bincount(flat, minlength=EXPERTS)
    offs = np.concatenate([[0], np.cumsum(counts)])

    pieces = []  # (expert, a, b)  [a:b) into the expert-major order
    for e in range(EXPERTS):
        a, b = int(offs[e]), int(offs[e + 1])
        # balanced splits for oversize experts: halving beats peeling
        # (CAP, tiny-remainder) since slot capacity = its group max
        while b - a > 2 * CAP:
            pieces.append((e, a, a + CAP))
            a += CAP
        if b - a > CAP:
            m = (b - a + 1) // 2
            pieces.append((e, a, a + m))
            a += m
        if b > a:
            pieces.append((e, a, b))

    S8 = max(1, math.ceil(len(pieces) / NCORES))

    def _sum_caps(sz):
        ss = sorted(sz, reverse=True) + [0] * (NCORES * S8 - len(sz))
        return sum(max(ss[NCORES * j: NCORES * (j + 1)]) for j in range(S8))

    # spend spare cells on midpoint splits of whichever piece lowers the
    # sum of per-slot capacities (= compute) the most
    while len(pieces) < NCORES * S8:
        sz = [b - a for _, a, b in pieces]
        base = _sum_caps(sz)
        best = None
        for i, n in enumerate(sz):
            if n < 2:
                continue
            cand = _sum_caps(sz[:i] + sz[i + 1:] + [n // 2, n - n // 2])
            if cand < base and (best is None or cand < best[0]):
                best = (cand, i)
        if best is None:
            break
        e, a, b = pieces.pop(best[1])
        m = (b - a) // 2
        pieces += [(e, a, a + m), (e, a + m, b)]

    pieces += [(-1, 0, 0)] * (NCORES * S8 - len(pieces))   # dummy cells
    pieces.sort(key=lambda p: p[1] - p[2])                 # size descending
    caps = [max(p[2] - p[1] for p in pieces[NCORES * j: NCORES * (j + 1)])
            for j in range(S8)]

    # shared tokens: contiguous ranges, T/NCORES per core in S16 slots
    per_core = math.ceil(T / NCORES)
    S16 = max(1, math.ceil(per_core / CAP))
    shared = []  # (a, b) token range per (core, slot)
    for c in range(NCORES):
        t0, t1 = min(c * per_core, T), min((c + 1) * per_core, T)
        for j in range(S16):
            a = min(t0 + j * CAP, t1)
            shared.append((a, min(a + CAP, t1)))
    return pieces, caps, shared, S16, order, tok_of


# ----------------------------------------------------------- device program

def _build_program(caps, S16):
    import concourse.mybir as mybir
    import concourse.tile as tile
    from concourse import bacc

    f32 = mybir.dt.float32
    fp8 = mybir.dt.float8e4
    fp16 = mybir.dt.float16

    S8 = len(caps)
    M8 = sum(caps)
    M16 = S16 * CAP
    M = M8 + M16

    nc = bacc.Bacc("TRN2", target_bir_lowering=False, debug=False,
                   enable_asserts=False, num_devices=NCORES)
    # xT8 spans ALL columns (routed + shared): layer 1 runs fp8-DoubleRow
    # for every slot; only the shared slot's layer 2 stays fp16.
    xT8 = nc.dram_tensor("xT8", [KT, 128, M], fp8, kind="ExternalInput").ap()
    w1s8 = nc.dram_tensor("w1s8", [S8 + S16, KT, 128, KT, 128], fp8, kind="ExternalInput").ap()
    w2s8 = nc.dram_tensor("w2s8", [S8 + S16, KT, 128, KT, 128], fp8, kind="ExternalInput").ap()
    b1s = nc.dram_tensor("b1s", [S8 + S16, 128, KT], f32, kind="ExternalInput").ap()
    b2s = nc.dram_tensor("b2s", [S8 + S16, 128, KT], f32, kind="ExternalInput").ap()
    bf16 = mybir.dt.bfloat16
    yT = nc.dram_tensor("yT", [KT, 128, M], bf16, kind="ExternalOutput").ap()

    Gelu = mybir.ActivationFunctionType.Gelu
    DR = mybir.MatmulPerfMode.DoubleRow

    with tile.TileContext(nc) as tc:
        with (
            tc.tile_pool(name="xa", bufs=3) as xa,
            tc.tile_pool(name="w1p", bufs=6) as w1p,
            tc.tile_pool(name="w2p", bufs=6) as w2p,
            tc.tile_pool(name="hp", bufs=3) as hp,
            tc.tile_pool(name="yo", bufs=6) as yo,
            tc.tile_pool(name="bp", bufs=8) as bp,
            tc.tile_pool(name="ps", bufs=8, space="PSUM") as ps,
        ):
            def slot(s, col0, cap, w2src, w2idx, h_dt, kstep2, s2,
                     first=False):
                pm2 = DR if kstep2 == 2 else None
                # ragged chunks: full 512s plus an arbitrary-width tail —
                # matmul free-dims need no alignment, so slot capacity can
                # match its largest piece exactly
                chunks = [(o, min(CHUNK, cap - o)) for o in range(0, cap, CHUNK)]
                w1_pre = None
                if first:
                    # first w1 load leads everything on the Sync queue so
                    # the first matmul isn't gated on it
                    w1_pre = w1p.tile([128, KT, 128], fp8, tag="w1",
                                      name="w1t")
                    nc.sync.dma_start(out=w1_pre[:, :, :], in_=w1s8[s, 0])
                xc = [xa.tile([128, KT, CHUNK], fp8, tag="x", name="xt")
                      for _ in chunks]
                if first:
                    # k-outer/c-inner to match the matmul consumption order
                    # (the k0 pair needs BOTH chunks first), split across
                    # the still-idle Scalar HWDGE + SWDGE queues
                    i = 0
                    for k in range(KT):
                        for ci, (off, n) in enumerate(chunks):
                            eng = nc.scalar if i % 2 else nc.gpsimd
                            eng.dma_start(
                                out=xc[ci][:, k, :n],
                                in_=xT8[k, :, col0 + off: col0 + off + n])
                            i += 1
                else:
                    # SWDGE: keeps HWDGE free for the slot's weight loads
                    for ci, (off, n) in enumerate(chunks):
                        for k in range(KT):
                            nc.gpsimd.dma_start(
                                out=xc[ci][:, k, :n],
                                in_=xT8[k, :, col0 + off: col0 + off + n])

                b1t = bp.tile([128, KT], f32, tag="b1", name="b1t")
                nc.sync.dma_start(out=b1t[:, :], in_=b1s[s])
                b2t = bp.tile([128, KT], f32, tag="b2", name="b2t")
                nc.sync.dma_start(out=b2t[:, :], in_=b2s[s])

                hc = [hp.tile([128, KT, CHUNK], h_dt, tag=f"h{kstep2}",
                              name=f"h{ci}") for ci in range(len(chunks))]

                # layer 1 (always fp8 DoubleRow): h = gelu((x @ w1)*s1 + b1)
                for io in range(KT):
                    if io == 0 and w1_pre is not None:
                        w1t = w1_pre
                    else:
                        w1t = w1p.tile([128, KT, 128], fp8, tag="w1",
                                       name="w1t")
                        nc.sync.dma_start(out=w1t[:, :, :], in_=w1s8[s, io])
                    pts = [ps.tile([128, CHUNK], f32, tag="ps", name="pt")
                           for _ in chunks]
                    # k outer, c inner: one weight (pair) load covers all
                    # chunks so LDWEIGHTS hides under the streaming
                    for k in range(0, KT, 2):
                        for ci, (off, n) in enumerate(chunks):
                            nc.tensor.matmul(pts[ci][:, :n],
                                             w1t[:, k:k + 2, :],
                                             xc[ci][:, k:k + 2, :n],
                                             start=(k == 0),
                                             stop=(k + 2 >= KT),
                                             perf_mode=DR)
                    for ci, (off, n) in enumerate(chunks):
                        nc.scalar.activation(hc[ci][:, io, :n],
                                             pts[ci][:, :n],
                                             Gelu, bias=b1t[:, io:io + 1],
                                             scale=1.0 / (WSCALE * XSCALE))

                # layer 2: y = (h @ w2) * s2 + b2
                for io in range(KT):
                    w2t = w2p.tile([128, KT, 128], h_dt, tag=f"w2{kstep2}",
                                   name="w2t")
                    nc.sync.dma_start(out=w2t[:, :, :], in_=w2src[w2idx, io])
                    pts = [ps.tile([128, CHUNK], f32, tag="ps", name="pt")
                           for _ in chunks]
                    for k in range(0, KT, kstep2):
                        for ci, (off, n) in enumerate(chunks):
                            nc.tensor.matmul(pts[ci][:, :n],
                                             w2t[:, k:k + kstep2, :],
                                             hc[ci][:, k:k + kstep2, :n],
                                             start=(k == 0),
                                             stop=(k + kstep2 >= KT),
                                             perf_mode=pm2)
                    # one bf16 y tile + one store per io (both chunks):
                    # halves store count and semaphore traffic so the
                    # DVE <-> store recycle loop stays ahead of the PE
                    yt = yo.tile([128, CAP], bf16, tag="y", name="yt")
                    for ci, (off, n) in enumerate(chunks):
                        # drain layer-2 PSUM on the (otherwise idle) Vector
                        # engine: y = psum * s2 + b2
                        nc.vector.tensor_scalar(
                            yt[:, off:off + n], pts[ci][:, :n], s2,
                            b2t[:, io:io + 1],
                            mybir.AluOpType.mult, mybir.AluOpType.add)
                    # y-stores ride the Scalar HWDGE queue so the Sync
                    # queue can issue the NEXT slot's weight loads
                    # without queuing behind gated stores
                    nc.scalar.dma_start(
                        out=yT[io, :, col0: col0 + cap], in_=yt[:, :cap])

            col0 = 0
            for s in range(S8):
                if caps[s]:
                    slot(s, col0, caps[s], w2s8, s, fp8, 2, 1.0 / WSCALE,
                         first=(s == 0))
                col0 += caps[s]
            for s in range(S16):
                slot(S8 + s, M8 + s * CAP, CAP, w2s8, S8 + s, fp8, 2,
                     1.0 / WSCALE)
    nc.compile()
    return nc


def _get_program(caps, S16):
    key = (tuple(caps), S16)
    if key not in _PROGRAM_CACHE:
        _PROGRAM_CACHE[key] = _build_program(tuple(caps), S16)
    return _PROGRAM_CACHE[key]


# ------------------------------------------------------------------ kernel

def _arrange_w(w):
    """[D, I] -> [io, p, ko, c] so each (slot, io) block DMAs contiguously
    into an SBUF tile laid out [partition, ko, col]."""
    return np.ascontiguousarray(
        w.reshape(KT, 128, KT, 128).transpose(2, 1, 0, 3))


def _q8(a, scale, dt):
    """Scale + saturate to TRN e4m3 range (+-240) before fp8 downcast."""
    return np.clip(a * np.float32(scale), -240.0, 240.0).astype(dt)


def kernel(x, sw1, sb1, sw2, sb2, rw1, rb1, rw2, rb2, wr, br, _trace=False):
    import ml_dtypes
    from concourse.bass_utils import run_bass_kernel_spmd

    fp8 = ml_dtypes.float8_e4m3

    x = np.asarray(x, dtype=np.float32)
    B, Sq, _ = x.shape
    T = B * Sq
    xf = np.ascontiguousarray(x.reshape(T, D))

    idx, vals = _route(xf, np.asarray(wr, np.float32), np.asarray(br, np.float32))
    pieces, caps, shared, S16, order, tok_of = _build_plan(T, idx)
    S8 = len(caps)
    col0s = np.concatenate([[0], np.cumsum(caps)]).astype(int)  # slot offsets
    M8, M16 = int(col0s[-1]), S16 * CAP
    M = M8 + M16

    rw1 = np.asarray(rw1, np.float32); rw2 = np.asarray(rw2, np.float32)
    rb1 = np.asarray(rb1, np.float32); rb2 = np.asarray(rb2, np.float32)
    sw1 = np.asarray(sw1, np.float32); sw2 = np.asarray(sw2, np.float32)
    sb1 = np.asarray(sb1, np.float32); sb2 = np.asarray(sb2, np.float32)

    w1a = [_q8(_arrange_w(rw1[e]), WSCALE, fp8) for e in range(EXPERTS)]
    w2a = [_q8(_arrange_w(rw2[e]), WSCALE, fp8) for e in range(EXPERTS)]
    sw1a = _q8(_arrange_w(sw1), WSCALE, fp8)      # shared L1 runs fp8-DR too
    sw2a = _q8(_arrange_w(sw2), WSCALE, fp8)      # shared L2 fp8-DR too
    b1a = [np.ascontiguousarray(rb1[e].reshape(KT, 128).T) for e in range(EXPERTS)]
    b2a = [np.ascontiguousarray(rb2[e].reshape(KT, 128).T) for e in range(EXPERTS)]
    sb1a = np.ascontiguousarray(sb1.reshape(KT, 128).T)
    sb2a = np.ascontiguousarray(sb2.reshape(KT, 128).T)

    xfT = np.ascontiguousarray(xf.T)                 # [D, T] f32
    xfT8 = _q8(xfT, XSCALE, fp8)                     # fp8 operand, all slots

    in_maps = []
    for core in range(NCORES):
        xT8c = np.zeros((D, M), dtype=fp8)
        w1c8 = np.zeros((S8 + S16, KT, 128, KT, 128), dtype=fp8)
        w2c8 = np.zeros((S8 + S16, KT, 128, KT, 128), dtype=fp8)
        b1c = np.zeros((S8 + S16, 128, KT), dtype=np.float32)
        b2c = np.zeros((S8 + S16, 128, KT), dtype=np.float32)
        for j in range(S8):
            e, a, b = pieces[NCORES * j + core]
            if e >= 0:
                c0 = col0s[j]
                xT8c[:, c0: c0 + (b - a)] = xfT8[:, tok_of[a:b]]
                w1c8[j] = w1a[e]; w2c8[j] = w2a[e]
                b1c[j] = b1a[e]; b2c[j] = b2a[e]
        for j in range(S16):
            a, b = shared[core * S16 + j]
            xT8c[:, M8 + j * CAP: M8 + j * CAP + (b - a)] = xfT8[:, a:b]
            w1c8[S8 + j] = sw1a; w2c8[S8 + j] = sw2a
            b1c[S8 + j] = sb1a; b2c[S8 + j] = sb2a
        in_maps.append({
            "xT8": xT8c.reshape(KT, 128, M),
            "w1s8": w1c8, "w2s8": w2c8,
            "b1s": b1c, "b2s": b2c,
        })

    nc = _get_program(caps, S16)
    res = run_bass_kernel_spmd(nc, in_maps, core_ids=list(range(NCORES)),
                               trace=_trace)
    kernel.last_result = res

    TK = T * TOPK
    gated = np.empty((TK, D), dtype=np.float32)   # expert-major rows
    shared_out = np.empty((T, D), dtype=np.float32)
    for core in range(NCORES):
        Y = np.asarray(res.results[core]["yT"], dtype=np.float32).reshape(D, M)
        for j in range(S8):
            e, a, b = pieces[NCORES * j + core]
            if e >= 0:
                c0 = col0s[j]
                gated[a:b] = Y[:, c0: c0 + (b - a)].T
        for j in range(S16):
            a, b = shared[core * S16 + j]
            shared_out[a:b] = Y[:, M8 + j * CAP: M8 + j * CAP + (b - a)].T

    g = vals.ravel()[order].astype(np.float32)
    gated *= g[:, None]
    ord2 = np.argsort(tok_of, kind="stable")      # token-major, expert asc
    routed = gated[ord2].reshape(T, TOPK, D).sum(axis=1, dtype=np.float32)

    out = shared_out + routed + xf
    return out.reshape(B, Sq, D).astype(np.float32)


kernel.last_result = None
